# revision 16
# baseline (speedup 1.0000x reference)
"""Trainium2 Bass kernel for nn_NetAE_35038343200962 (moe_routing).

Single-vector MoE forward:
  base MLP (2048->2048->2048) -> 16 experts (W_exp/W_v/W_k, 2048x2048 each)
  -> attention-score-weighted expert sum -> tower (2048->2048) -> last (2048->512).

~596MB of fp32 weights for one matvec chain => completely HBM-bound.
Strategy (8 cores, one trn2 chip, LNC1):
  - base/tower/last fully sharded (column/row split, 1/8 of each matrix per core)
  - experts: expert-parallel, 2 experts per core
  - 2 on-device AllReduces (8KB each): base-MLP partial sum, weighted expert sum
  - final 512-wide partials summed on host
Per-core HBM traffic: ~74.5MB, vs ~596MB single-core => ~8x.

Layout trick: activations are kept "partition-major" ([128, T] SBUF tiles with
element (p,t) = vec[t*128+p]) so they can feed matmuls as lhsT/rhs without any
on-device transposes.  Matvecs that must produce partition-major outputs use
the weights as the stationary operand (out [128,1] chunks, accumulated across
k-stripes on DVE since PSUM accumulation groups are per-bank); matvecs whose
output crosses DRAM (the two AllReduces) use the streaming form (out [1,N] on
one partition) with the weight columns pre-permuted on the host so that the
natural DRAM round-trip restores partition-major order.
"""

import sys

import numpy as np

for _p in ("/opt/trn_rl_repo", "/root/.axon_site/_ro/trn_rl_repo"):
    if _p not in sys.path:
        sys.path.append(_p)

import concourse.bacc as bacc
import concourse.mybir as mybir
import concourse.tile as tile
from concourse.bass_utils import run_bass_kernel_spmd

N_CORES = 8
D = 2048          # d_in = d_h = d_v = d_t
D_A = 256         # attention dim
D_OUT = 512
N_EXP = 16
E = N_EXP // N_CORES  # experts per core (2)
KT = D // 128     # 16 k-tiles of 128 along a 2048 contraction
F32 = mybir.dt.float32

# sigma: psum free index j -> original vector index, chosen so that dumping
# psum[0, :] to DRAM flat and re-loading as [128, 16] lands partition-major.
_SIGMA = (np.arange(D) % 16) * 128 + np.arange(D) // 16

# debug knobs
VARIANT = "full"   # "full" | "noar" (collectives replaced by local dram copies)
UPTO = 3           # 0=dma only, 1=+base, 2=+experts, 3=+tower (full)
EXPSUB = "full"    # "e" | "ek" | "ekdot" | "ekv" | "full"


def _pm(v):
    """vector [n*128] -> partition-major [128, n] (tile[p, t] = v[t*128+p])."""
    n = v.shape[0] // 128
    return np.ascontiguousarray(v.reshape(n, 128).T)


def _build_program():
    nc = bacc.Bacc(
        "TRN2", target_bir_lowering=False, debug=False, num_devices=N_CORES
    )

    # ---- per-core DRAM inputs ----
    xp = nc.dram_tensor("xp", [128, KT], F32, kind="ExternalInput")
    w1 = nc.dram_tensor("w1", [KT, 128, D_A], F32, kind="ExternalInput")
    b1p = nc.dram_tensor("b1p", [128, 2], F32, kind="ExternalInput")
    w2 = nc.dram_tensor("w2", [2, 128, D], F32, kind="ExternalInput")
    b2p = nc.dram_tensor("b2p", [128, KT], F32, kind="ExternalInput")
    wexp = nc.dram_tensor("wexp", [E, KT, 128, D], F32, kind="ExternalInput")
    bexpp = nc.dram_tensor("bexpp", [E, 128, KT], F32, kind="ExternalInput")
    wk = nc.dram_tensor("wk", [E, KT, 128, D_A], F32, kind="ExternalInput")
    bk = nc.dram_tensor("bk", [E, 1, D_A], F32, kind="ExternalInput")
    qv = nc.dram_tensor("qv", [1, D_A], F32, kind="ExternalInput")
    wv = nc.dram_tensor("wv", [E, KT, 128, D], F32, kind="ExternalInput")
    bv = nc.dram_tensor("bv", [E, 1, D], F32, kind="ExternalInput")
    wt = nc.dram_tensor("wt", [KT, 128, D_A], F32, kind="ExternalInput")
    btp = nc.dram_tensor("btp", [128, 2], F32, kind="ExternalInput")
    wl = nc.dram_tensor("wl", [2, 128, D_OUT], F32, kind="ExternalInput")
    yp = nc.dram_tensor("yp", [1, D_OUT], F32, kind="ExternalOutput")

    add = mybir.AluOpType.add
    mult = mybir.AluOpType.mult

    with tile.TileContext(nc) as tc:
        with (
            tc.tile_pool(name="wbig", bufs=10) as wbig,      # [128,2048] stripes
            tc.tile_pool(name="w256", bufs=6) as w256,       # [128,256] stripes
            tc.tile_pool(name="wtp", bufs=KT) as wtp,        # tower stripes, all resident
            tc.tile_pool(name="wlp", bufs=2) as wlp,         # last stripes
            tc.tile_pool(name="acts", bufs=1) as acts,       # activations/biases
            tc.tile_pool(name="row2048", bufs=2) as row2048, # transient [1,2048]
            # psA: "bigps" [1,2048] = 4 banks, "rowps" [1,512] = 1 bank.
            # psB: per-stripe W-stationary outputs, 2 slots (ping-pong so the
            # DVE cross-stripe accumulate overlaps the next stripe's matmuls).
            tc.tile_pool(name="psA", bufs=1, space="PSUM") as psA,
            tc.tile_pool(name="psB", bufs=2, space="PSUM") as psB,
            tc.tile_pool(name="dram", bufs=1, space="DRAM") as dram,
        ):
            # ---- constant loads ----
            x_sb = acts.tile([128, KT], F32)
            nc.sync.dma_start(x_sb[:], xp[:])
            b1_sb = acts.tile([128, 2], F32)
            nc.sync.dma_start(b1_sb[:], b1p[:])
            b2_sb = acts.tile([128, KT], F32)
            nc.sync.dma_start(b2_sb[:], b2p[:])
            bt_sb = acts.tile([128, 2], F32)
            nc.sync.dma_start(bt_sb[:], btp[:])
            q_sb = acts.tile([1, D_A], F32)
            nc.sync.dma_start(q_sb[:], qv[:])
            bexp_sb = []
            bk_sb = []
            bv_sb = []
            for i in range(E):
                be = acts.tile([128, KT], F32, name=f"bexp{i}")
                nc.sync.dma_start(be[:], bexpp[i])
                bexp_sb.append(be)
                bks = acts.tile([1, D_A], F32, name=f"bk{i}")
                nc.sync.dma_start(bks[:], bk[i])
                bk_sb.append(bks)
                bvs = acts.tile([1, D], F32, name=f"bv{i}")
                nc.sync.dma_start(bvs[:], bv[i])
                bv_sb.append(bvs)
            # preload the post-AllReduce tower/last weights so the tail is
            # compute-only
            wt_tiles = []
            for k in range(KT):
                w = wtp.tile([128, D_A], F32, tag="wt")
                nc.sync.dma_start(w[:], wt[k])
                wt_tiles.append(w)
            wl_tiles = []
            for j in range(2):
                w = wlp.tile([128, D_OUT], F32, tag="wl")
                nc.sync.dma_start(w[:], wl[j])
                wl_tiles.append(w)

            def allreduce(src_sb, tag):
                """SBUF [1,D] -> DRAM -> AllReduce -> SBUF [128,KT]."""
                cin = dram.tile([1, D], F32, name=f"ar{tag}_in")
                cout = dram.tile([1, D], F32, name=f"ar{tag}_out")
                nc.sync.dma_start(cin[:], src_sb[:])
                if VARIANT == "noar":
                    nc.gpsimd.dma_start(cout[:], cin[:])
                else:
                    nc.gpsimd.collective_compute(
                        "AllReduce",
                        add,
                        replica_groups=[list(range(N_CORES))],
                        ins=[cin.opt()],
                        outs=[cout.opt()],
                    )
                dst = acts.tile([128, KT], F32, name=f"ar{tag}_sb")
                nc.sync.dma_start(
                    dst[:], cout[:].rearrange("a (p t) -> (a p) t", p=128)
                )
                return dst

            # ================= base MLP =================
            if UPTO >= 1:
                # layer 1: W stationary -> partition-major out.
                # PSUM `start` clears has_written for the whole 2KB bank, so
                # per-column accumulation groups can't interleave in one
                # bank: each stripe's matmuls are single-mm groups,
                # accumulated across stripes on DVE in SBUF.
                out1_sb = acts.tile([128, 2], F32)
                for k in range(KT):
                    w = w256.tile([128, D_A], F32, tag="w256")
                    nc.sync.dma_start(w[:], w1[k])
                    ps = psB.tile([128, 2], F32, tag="wmps", name=f"w1ps{k}")
                    for j in range(2):
                        nc.tensor.matmul(
                            ps[:, j : j + 1],
                            w[:, j * 128 : (j + 1) * 128],
                            x_sb[:, k : k + 1],
                            start=True,
                            stop=True,
                        )
                    if k == 0:
                        nc.vector.tensor_copy(out1_sb[:], ps[:])
                    else:
                        nc.vector.tensor_add(out1_sb[:], out1_sb[:], ps[:])
                nc.vector.tensor_add(out1_sb[:], out1_sb[:], b1_sb[:])
                nc.vector.tensor_relu(out1_sb[:], out1_sb[:])

                # layer 2: streaming, sigma-permuted cols
                p2_ps = psA.tile([1, D], F32, tag="bigps")
                for k in range(2):
                    w = wbig.tile([128, D], F32, tag="wstripe")
                    nc.sync.dma_start(w[:], w2[k])
                    for c in range(4):
                        nc.tensor.matmul(
                            p2_ps[0:1, c * 512 : (c + 1) * 512],
                            out1_sb[:, k : k + 1],
                            w[:, c * 512 : (c + 1) * 512],
                            start=(k == 0),
                            stop=(k == 1),
                        )
                p2_sb = row2048.tile([1, D], F32, tag="row2048")
                nc.vector.tensor_copy(p2_sb[:], p2_ps[:])
                out_sb = allreduce(p2_sb, "1")
                nc.vector.tensor_add(out_sb[:], out_sb[:], b2_sb[:])
                nc.vector.tensor_relu(out_sb[:], out_sb[:])
            else:
                for k in range(KT):
                    w = w256.tile([128, D_A], F32, tag="w256", name=f"dw1{k}")
                    nc.sync.dma_start(w[:], w1[k])
                for k in range(2):
                    w = wbig.tile([128, D], F32, tag="wstripe", name=f"dw2{k}")
                    nc.sync.dma_start(w[:], w2[k])
                out_sb = x_sb

            # ================= experts =================
            res_sb = acts.tile([1, D], F32)
            if UPTO >= 2:
                for i in range(E):
                    # e_i = relu(out @ W_exp[i] + b_exp[i]), partition-major
                    e_sb = acts.tile([128, KT], F32, name=f"e{i}")
                    for k in range(KT):
                        w = wbig.tile([128, D], F32, tag="wstripe")
                        nc.sync.dma_start(w[:], wexp[i, k])
                        ps = psB.tile(
                            [128, KT], F32, tag="wmps", name=f"eps{i}_{k}"
                        )
                        for c in range(KT):
                            nc.tensor.matmul(
                                ps[:, c : c + 1],
                                w[:, c * 128 : (c + 1) * 128],
                                out_sb[:, k : k + 1],
                                start=True,
                                stop=True,
                            )
                        if k == 0:
                            nc.vector.tensor_copy(e_sb[:], ps[:])
                        else:
                            nc.vector.tensor_add(e_sb[:], e_sb[:], ps[:])
                    nc.vector.tensor_add(e_sb[:], e_sb[:], bexp_sb[i][:])
                    nc.vector.tensor_relu(e_sb[:], e_sb[:])

                    # k_i = e_i @ W_k[i] + b_k[i]; alpha_i = k_i . q
                    if EXPSUB == "e":
                        for k in range(KT):
                            w = w256.tile(
                                [128, D_A], F32, tag="w256", name=f"xwk{i}_{k}"
                            )
                            nc.sync.dma_start(w[:], wk[i, k])
                        alpha = None
                    else:
                        k_ps = psA.tile([1, D_A], F32, tag="rowps")
                        for k in range(KT):
                            w = w256.tile([128, D_A], F32, tag="w256")
                            nc.sync.dma_start(w[:], wk[i, k])
                            nc.tensor.matmul(
                                k_ps[0:1, :],
                                e_sb[:, k : k + 1],
                                w[:],
                                start=(k == 0),
                                stop=(k == KT - 1),
                            )
                        kb_sb = acts.tile([1, D_A], F32, name=f"kb{i}")
                        nc.vector.tensor_add(kb_sb[:], k_ps[:], bk_sb[i][:])
                        if EXPSUB == "ek":
                            alpha = None
                        else:
                            # (tensor_tensor_reduce is a custom DVE op whose
                            # ucode table doesn't ship on this path - use
                            # native mul + reduce instead)
                            dot_sb = acts.tile([1, D_A], F32, name=f"dot{i}")
                            alpha = acts.tile([1, 1], F32, name=f"alpha{i}")
                            nc.vector.tensor_mul(dot_sb[:], kb_sb[:], q_sb[:])
                            nc.vector.reduce_sum(
                                alpha[:], dot_sb[:], mybir.AxisListType.X
                            )

                    # v_i = e_i @ W_v[i] (sigma-permuted cols);
                    # res += alpha_i * (v_i + b_v[i])
                    if EXPSUB in ("e", "ek", "ekdot"):
                        for k in range(KT):
                            w = wbig.tile(
                                [128, D], F32, tag="wstripe", name=f"xwv{i}_{k}"
                            )
                            nc.sync.dma_start(w[:], wv[i, k])
                        if i == 0:
                            nc.vector.tensor_copy(res_sb[:], bv_sb[0][:])
                        continue
                    v_ps = psA.tile([1, D], F32, tag="bigps")
                    for k in range(KT):
                        w = wbig.tile([128, D], F32, tag="wstripe")
                        nc.sync.dma_start(w[:], wv[i, k])
                        for c in range(4):
                            nc.tensor.matmul(
                                v_ps[0:1, c * 512 : (c + 1) * 512],
                                e_sb[:, k : k + 1],
                                w[:, c * 512 : (c + 1) * 512],
                                start=(k == 0),
                                stop=(k == KT - 1),
                            )
                    vb_sb = row2048.tile([1, D], F32, tag="row2048")
                    nc.vector.tensor_add(vb_sb[:], v_ps[:], bv_sb[i][:])
                    if EXPSUB == "ekv":
                        if i == 0:
                            nc.vector.tensor_copy(res_sb[:], vb_sb[:])
                        else:
                            nc.vector.tensor_add(res_sb[:], res_sb[:], vb_sb[:])
                    elif i == 0:
                        nc.vector.tensor_scalar_mul(res_sb[:], vb_sb[:], alpha[:])
                    else:
                        nc.vector.scalar_tensor_tensor(
                            res_sb[:], vb_sb[:], alpha[:], res_sb[:], mult, add
                        )
            else:
                for i in range(E):
                    for k in range(KT):
                        w = wbig.tile(
                            [128, D], F32, tag="wstripe", name=f"dwe{i}_{k}"
                        )
                        nc.sync.dma_start(w[:], wexp[i, k])
                    for k in range(KT):
                        w = w256.tile(
                            [128, D_A], F32, tag="w256", name=f"dwk{i}_{k}"
                        )
                        nc.sync.dma_start(w[:], wk[i, k])
                    for k in range(KT):
                        w = wbig.tile(
                            [128, D], F32, tag="wstripe", name=f"dwv{i}_{k}"
                        )
                        nc.sync.dma_start(w[:], wv[i, k])
                nc.vector.tensor_copy(res_sb[:], bv_sb[0][:])

            # ================= AllReduce + tower + last =================
            res_p = allreduce(res_sb, "2")
            if UPTO >= 3:
                t_sb = acts.tile([128, 2], F32)
                for k in range(KT):
                    ps = psB.tile([128, 2], F32, tag="wmps", name=f"tps{k}")
                    for j in range(2):
                        nc.tensor.matmul(
                            ps[:, j : j + 1],
                            wt_tiles[k][:, j * 128 : (j + 1) * 128],
                            res_p[:, k : k + 1],
                            start=True,
                            stop=True,
                        )
                    if k == 0:
                        nc.vector.tensor_copy(t_sb[:], ps[:])
                    else:
                        nc.vector.tensor_add(t_sb[:], t_sb[:], ps[:])
                nc.vector.tensor_add(t_sb[:], t_sb[:], bt_sb[:])
                nc.vector.tensor_relu(t_sb[:], t_sb[:])

                o_ps = psA.tile([1, D_OUT], F32, tag="rowps")
                for j in range(2):
                    nc.tensor.matmul(
                        o_ps[0:1, :],
                        t_sb[:, j : j + 1],
                        wl_tiles[j][:],
                        start=(j == 0),
                        stop=(j == 1),
                    )
                o_sb = acts.tile([1, D_OUT], F32)
                nc.vector.tensor_copy(o_sb[:], o_ps[:])
            else:
                o_sb = acts.tile([1, D_OUT], F32)
                nc.vector.tensor_copy(o_sb[:], res_sb[0:1, 0:D_OUT])
                res_p[:]  # keep the reload referenced
            nc.sync.dma_start(yp[:], o_sb[:])

    nc.compile()
    return nc


_NC_CACHE = None


def _get_nc():
    global _NC_CACHE
    if _NC_CACHE is None:
        _NC_CACHE = _build_program()
    return _NC_CACHE


def make_in_maps(
    x, W1, b1, W2, b2, W_exp, b_exp, W_v, b_v, W_k, b_k, W_q, b_q, W_t, b_t,
    W_l, b_l, task_id,
):
    """Host-side shard/permute of the full inputs into 8 per-core input maps."""
    f = np.float32
    x, W1, b1, W2, b2, W_exp, b_exp, W_v, b_v, W_k, b_k, W_q, b_q, W_t, b_t, W_l = (
        np.asarray(a, f)
        for a in (
            x, W1, b1, W2, b2, W_exp, b_exp, W_v, b_v, W_k, b_k, W_q, b_q,
            W_t, b_t, W_l,
        )
    )
    tid = int(task_id)
    q = W_q[tid, tid, :] + b_q[tid, :]

    xpm = _pm(x)
    b2pm = _pm(b2)
    W2s = W2[:, _SIGMA]
    Wvs = W_v[:, :, _SIGMA]
    bvs = b_v[:, _SIGMA]

    in_maps = []
    for c in range(N_CORES):
        cs = slice(c * D_A, (c + 1) * D_A)        # 256-wide base/tower slice
        es = slice(c * E, (c + 1) * E)            # expert slice
        ls = slice(c * D_A, (c + 1) * D_A)        # last-layer row slice
        in_maps.append(
            {
                "xp": xpm,
                "w1": np.ascontiguousarray(W1[:, cs]).reshape(KT, 128, D_A),
                "b1p": _pm(np.asarray(b1[cs], f)),
                "w2": np.ascontiguousarray(W2s[cs, :]).reshape(2, 128, D),
                "b2p": b2pm,
                "wexp": np.ascontiguousarray(W_exp[es]).reshape(E, KT, 128, D),
                "bexpp": np.stack([_pm(b_exp[c * E + i]) for i in range(E)]),
                "wk": np.ascontiguousarray(W_k[es]).reshape(E, KT, 128, D_A),
                "bk": np.ascontiguousarray(b_k[es]).reshape(E, 1, D_A),
                "qv": q.reshape(1, D_A),
                "wv": np.ascontiguousarray(Wvs[es]).reshape(E, KT, 128, D),
                "bv": np.ascontiguousarray(bvs[es]).reshape(E, 1, D),
                "wt": np.ascontiguousarray(W_t[:, cs]).reshape(KT, 128, D_A),
                "btp": _pm(np.asarray(b_t[cs], f)),
                "wl": np.ascontiguousarray(W_l[ls, :]).reshape(2, 128, D_OUT),
            }
        )
    return in_maps


def kernel(**inputs):
    nc = _get_nc()
    in_maps = make_in_maps(**inputs)
    res = run_bass_kernel_spmd(nc, in_maps, list(range(N_CORES)))
    partial = np.stack(
        [res.results[c]["yp"].reshape(D_OUT) for c in range(N_CORES)]
    )
    out = partial.sum(axis=0) + np.asarray(inputs["b_l"], np.float32)
    return out.astype(np.float32)
